# revision 6
# baseline (speedup 1.0000x reference)
"""CTRNN Bass/Tile kernel for Trainium2 (8 NeuronCores, SPMD data-parallel over batch).

Problem: h_t = ReLU((1-a)*h + a*(W @ [x_t, h] + b)),  T=512, B=64, I=256, H=1024.

Strategy (per core, B_local = 8 batches):
  - Host prep: MT = ((1-a)I + a*Wh)^T, WxT = a*Wx^T, ab = a*b, h0T = h0^T.
  - Device: xa = x @ WxT + ab precomputed per 64-step block with PE GEMMs
    (x transposed on-chip via PE transpose). Stored as (128 rows = 16 steps
    x 8 batch, 1024) tiles.
  - Scan: h kept TRANSPOSED in SBUF as hT (128 partitions = hidden chunk,
    8 k-chunks x 8 batch on free). Per step:
      psum(8,512) x2 = sum_c  hT[:,c,:].T @ MT[:,c,half]   (h is the
      stationary operand -> cheap 8-col LDWEIGHTS, MT is the moving operand)
      + selector-matmul injecting the xa row-block (keeps every compute op
      at base partition 0),
      ReLU psum -> h_nat (8,1024) on ScalarE,
      8 PE transposes h_nat -> psum(128,64) -> copy -> hT for next step,
      DMA h_nat -> y[t].
"""

import numpy as np
from contextlib import ExitStack

import concourse.bass as bass
import concourse.bacc as bacc
import concourse.mybir as mybir
import concourse.tile as tile
from concourse.bass_utils import run_bass_kernel_spmd
from concourse.masks import make_identity

F32 = mybir.dt.float32

T, B, I, H = 512, 64, 256, 1024
ALPHA = 10.0 / 100.0
NCORES = 8
BL = B // NCORES          # 8 batches per core
P = 128
KC = H // P               # 8 hidden chunks
IC = I // P               # 2 input chunks
NHALF = 2                 # 1024 = 2 x 512 (fp32 moving-operand max)
STEPS_PER_ROWTILE = P // BL   # 16 timesteps per 128-row tile
T_BLK = 64                # timesteps per xa block (4 row-tiles of 128)


def build_nc(t_total: int = T) -> bass.Bass:
    assert t_total % T_BLK == 0
    nc = bacc.Bacc()

    x_d = nc.dram_tensor("x", [t_total * BL, I], F32, kind="ExternalInput")
    h0T_d = nc.dram_tensor("h0T", [H, BL], F32, kind="ExternalInput")
    MT_d = nc.dram_tensor("MT", [H, H], F32, kind="ExternalInput")
    WxT_d = nc.dram_tensor("WxT", [I, H], F32, kind="ExternalInput")
    ab_d = nc.dram_tensor("ab", [H], F32, kind="ExternalInput")
    y_d = nc.dram_tensor("y", [t_total * BL, H], F32, kind="ExternalOutput")

    with ExitStack() as ctx:
        tc = ctx.enter_context(tile.TileContext(nc))
        consts = ctx.enter_context(tc.tile_pool(name="consts", bufs=1))
        xpool = ctx.enter_context(tc.tile_pool(name="xpool", bufs=3))
        xtpool = ctx.enter_context(tc.tile_pool(name="xtpool", bufs=3))
        xapool = ctx.enter_context(tc.tile_pool(name="xapool", bufs=8))
        hnat_pool = ctx.enter_context(tc.tile_pool(name="hnat", bufs=4))
        ht_pool = ctx.enter_context(tc.tile_pool(name="ht", bufs=2))
        ps_pre = ctx.enter_context(tc.tile_pool(name="ps_pre", bufs=4, space="PSUM"))
        ps_tr = ctx.enter_context(tc.tile_pool(name="ps_tr", bufs=2, space="PSUM"))
        ps_big = ctx.enter_context(tc.tile_pool(name="ps_big", bufs=2, space="PSUM"))

        # ---- persistent constants ----
        MT_s = consts.tile([P, KC, H], F32)
        nc.sync.dma_start(MT_s[:], MT_d[:, :].rearrange("(c p) j -> p c j", p=P))
        WxT_s = consts.tile([P, IC, H], F32)
        nc.sync.dma_start(WxT_s[:], WxT_d[:, :].rearrange("(c p) j -> p c j", p=P))
        ab_bc = consts.tile([P, H], F32)
        nc.sync.dma_start(ab_bc[:], ab_d[None, :].to_broadcast((P, H)))
        ident = consts.tile([P, P], F32)
        make_identity(nc, ident[:])

        hT_cur = ht_pool.tile([P, KC * BL], F32)
        nc.sync.dma_start(
            hT_cur[:].rearrange("p (c b) -> p c b", c=KC),
            h0T_d[:, :].rearrange("(c p) b -> p c b", p=P),
        )

        n_blocks = t_total // T_BLK
        rowtiles_per_blk = T_BLK // STEPS_PER_ROWTILE  # 4

        for blk in range(n_blocks):
            # ---- xa precompute for this block of 64 timesteps ----
            xa_tiles = []
            for sb in range(rowtiles_per_blk):
                row0 = (blk * T_BLK + sb * STEPS_PER_ROWTILE) * BL
                xt = xpool.tile([P, I], F32)
                nc.sync.dma_start(xt[:], x_d[row0:row0 + P, :])
                xT = xtpool.tile([P, IC, P], F32)
                for ic in range(IC):
                    pt = ps_big.tile([P, P], F32, tag="big")
                    nc.tensor.transpose(pt[:], xt[:, ic * P:(ic + 1) * P], ident[:])
                    nc.vector.tensor_copy(xT[:, ic, :], pt[:])
                xa_t = xapool.tile([P, H], F32)
                for nh in range(NHALF):
                    pp = ps_big.tile([P, 512], F32, tag="big")
                    for ic in range(IC):
                        nc.tensor.matmul(
                            pp[:, :],
                            xT[:, ic, :],
                            WxT_s[:, ic, nh * 512:(nh + 1) * 512],
                            start=(ic == 0),
                            stop=(ic == IC - 1),
                        )
                    # evict psum -> sbuf, adding the (broadcast) bias
                    nc.vector.tensor_tensor(
                        xa_t[:, nh * 512:(nh + 1) * 512],
                        pp[:, :],
                        ab_bc[:, nh * 512:(nh + 1) * 512],
                        mybir.AluOpType.add,
                    )
                xa_tiles.append(xa_t)

            # ---- the sequential scan over 64 timesteps ----
            for tt in range(T_BLK):
                t = blk * T_BLK + tt
                xa_t = xa_tiles[tt // STEPS_PER_ROWTILE]
                r0 = (tt % STEPS_PER_ROWTILE) * BL

                pss = []
                for nh in range(NHALF):
                    ps = ps_pre.tile([BL, 512], F32, tag="pre")
                    for c in range(KC):
                        nc.tensor.matmul(
                            ps[:, :],
                            hT_cur[:, c * BL:(c + 1) * BL],
                            MT_s[:, c, nh * 512:(nh + 1) * 512],
                            start=(c == 0),
                            stop=False,
                        )
                    # inject xa row-block via selector matmul:
                    # out[m,n] += sum_p I[p, r0+m] * xa[p, n] = xa[r0+m, n]
                    nc.tensor.matmul(
                        ps[:, :],
                        ident[:, r0:r0 + BL],
                        xa_t[:, nh * 512:(nh + 1) * 512],
                        start=False,
                        stop=True,
                    )
                    pss.append(ps)

                h_nat = hnat_pool.tile([BL, H], F32)
                for nh in range(NHALF):
                    nc.scalar.activation(
                        h_nat[:, nh * 512:(nh + 1) * 512],
                        pss[nh][:, :],
                        mybir.ActivationFunctionType.Relu,
                    )

                # transpose h_nat back into hT layout for the next step
                pt = ps_tr.tile([P, KC * BL], F32, tag="tr")
                for c in range(KC):
                    nc.tensor.transpose(
                        pt[:, c * BL:(c + 1) * BL],
                        h_nat[:, c * P:(c + 1) * P],
                        ident[:BL, :BL],
                    )
                hT_new = ht_pool.tile([P, KC * BL], F32)
                nc.vector.tensor_copy(hT_new[:], pt[:])

                nc.sync.dma_start(y_d[t * BL:(t + 1) * BL, :], h_nat[:])
                hT_cur = hT_new

    if not nc.is_finalized():
        nc.finalize()
    return nc


def _host_prep(x, h0, W, b):
    """Shard + pre-transform inputs for the 8 cores."""
    a = np.float32(ALPHA)
    Wx = W[:, :I]                    # (H, I)
    Wh = W[:, I:]                    # (H, H)
    MT = ((1.0 - a) * np.eye(H, dtype=np.float32) + a * Wh).T.copy()  # (H, H) = M'^T
    WxT = (a * Wx).T.copy()          # (I, H)
    ab = (a * b).astype(np.float32)  # (H,)

    in_maps = []
    for core in range(NCORES):
        bs = slice(core * BL, (core + 1) * BL)
        x_local = np.ascontiguousarray(x[:, bs, :]).reshape(T * BL, I)
        h0T = np.ascontiguousarray(h0[bs, :].T)  # (H, BL)
        in_maps.append({
            "x": x_local.astype(np.float32),
            "h0T": h0T.astype(np.float32),
            "MT": MT,
            "WxT": WxT,
            "ab": ab,
        })
    return in_maps


def run(x, h0, W, b, trace=False, **spmd_kwargs):
    x = np.asarray(x, dtype=np.float32)
    h0 = np.asarray(h0, dtype=np.float32)
    W = np.asarray(W, dtype=np.float32)
    b = np.asarray(b, dtype=np.float32)

    nc = build_nc(T)
    in_maps = _host_prep(x, h0, W, b)
    res = run_bass_kernel_spmd(
        nc, in_maps, list(range(NCORES)), trace=trace, **spmd_kwargs
    )

    y = np.empty((T, B, H), dtype=np.float32)
    for core in range(NCORES):
        y[:, core * BL:(core + 1) * BL, :] = res.results[core]["y"].reshape(T, BL, H)
    h_final = y[-1].copy()
    return y, h_final, res


def kernel(x, h0, W, b):
    y, h_final, _ = run(x, h0, W, b)
    return y, h_final


# revision 9
# speedup vs baseline: 5.8740x; 5.8740x over previous
"""CTRNN Bass/Tile kernel for Trainium2 (8 NeuronCores, SPMD data-parallel over batch).

Problem: h_t = ReLU((1-a)*h + a*(W @ [x_t, h] + b)),  T=512, B=64, I=256, H=1024.

Strategy (per core, B_local = 8 batches):
  - Host prep: MT = ((1-a)I + a*Wh)^T, WxT = a*Wx^T, ab = a*b, h0T = h0^T.
  - Device: xa = x @ WxT + ab precomputed per 64-step block with PE GEMMs
    (x transposed on-chip via PE transpose). Stored as (128 rows = 16 steps
    x 8 batch, 1024) tiles.
  - Scan: h kept TRANSPOSED in SBUF as hT (128 partitions = hidden chunk,
    8 k-chunks x 8 batch on free). Per step:
      psum(8,512) x2 = sum_c  hT[:,c,:].T @ MT[:,c,half]   (h is the
      stationary operand -> cheap 8-col LDWEIGHTS, MT is the moving operand)
      + selector-matmul injecting the xa row-block (keeps every compute op
      at base partition 0),
      ReLU psum -> h_nat (8,1024) on ScalarE,
      8 PE transposes h_nat -> psum(128,64) -> copy -> hT for next step,
      DMA h_nat -> y[t].
"""

import numpy as np
from contextlib import ExitStack

import concourse.bass as bass
import concourse.bacc as bacc
import concourse.mybir as mybir
import concourse.tile as tile
from concourse.bass_utils import run_bass_kernel_spmd
from concourse.masks import make_identity

F32 = mybir.dt.float32

T, B, I, H = 512, 64, 256, 1024
ALPHA = 10.0 / 100.0
NCORES = 8
BL = B // NCORES          # 8 batches per core
P = 128
KC = H // P               # 8 hidden chunks
IC = I // P               # 2 input chunks
NHALF = 2                 # 1024 = 2 x 512 (fp32 moving-operand max)
STEPS_PER_ROWTILE = P // BL   # 16 timesteps per 128-row tile
T_BLK = 64                # timesteps per xa block (4 row-tiles of 128)


def build_nc(t_total: int = T, repeat: int = 1) -> bass.Bass:
    assert t_total % T_BLK == 0
    nc = bacc.Bacc()

    x_d = nc.dram_tensor("x", [t_total * BL, I], F32, kind="ExternalInput")
    h0T_d = nc.dram_tensor("h0T", [H, BL], F32, kind="ExternalInput")
    MT_d = nc.dram_tensor("MT", [H, H], F32, kind="ExternalInput")
    WxT_d = nc.dram_tensor("WxT", [I, H], F32, kind="ExternalInput")
    ab_d = nc.dram_tensor("ab", [H], F32, kind="ExternalInput")
    y_d = nc.dram_tensor("y", [t_total * BL, H], F32, kind="ExternalOutput")

    with ExitStack() as ctx:
        tc = ctx.enter_context(tile.TileContext(nc))
        consts = ctx.enter_context(tc.tile_pool(name="consts", bufs=1))
        xpool = ctx.enter_context(tc.tile_pool(name="xpool", bufs=3))
        xtpool = ctx.enter_context(tc.tile_pool(name="xtpool", bufs=3))
        xapool = ctx.enter_context(tc.tile_pool(name="xapool", bufs=8))
        hnat_pool = ctx.enter_context(tc.tile_pool(name="hnat", bufs=4))
        ht_pool = ctx.enter_context(tc.tile_pool(name="ht", bufs=2))
        ps_pre = ctx.enter_context(tc.tile_pool(name="ps_pre", bufs=4, space="PSUM"))
        ps_tr = ctx.enter_context(tc.tile_pool(name="ps_tr", bufs=2, space="PSUM"))
        ps_big = ctx.enter_context(tc.tile_pool(name="ps_big", bufs=2, space="PSUM"))

        # ---- persistent constants ----
        MT_s = consts.tile([P, KC, H], F32)
        nc.sync.dma_start(MT_s[:], MT_d[:, :].rearrange("(c p) j -> p c j", p=P))
        WxT_s = consts.tile([P, IC, H], F32)
        nc.sync.dma_start(WxT_s[:], WxT_d[:, :].rearrange("(c p) j -> p c j", p=P))
        ab_bc = consts.tile([P, H], F32)
        nc.sync.dma_start(ab_bc[:], ab_d[None, :].to_broadcast((P, H)))
        ident = consts.tile([P, P], F32)
        make_identity(nc, ident[:])

        n_blocks = t_total // T_BLK
        rowtiles_per_blk = T_BLK // STEPS_PER_ROWTILE  # 4

        for rep in range(repeat):
            hT_cur = ht_pool.tile([P, KC * BL], F32)
            nc.sync.dma_start(
                hT_cur[:].rearrange("p (c b) -> p c b", c=KC),
                h0T_d[:, :].rearrange("(c p) b -> p c b", p=P),
            )
            _scan_body(nc, ht_pool, xpool, xtpool, xapool, hnat_pool,
                       ps_pre, ps_tr, ps_big,
                       MT_s, WxT_s, ab_bc, ident, hT_cur,
                       x_d, y_d, n_blocks, rowtiles_per_blk)

    if not nc.is_finalized():
        nc.finalize()
    return nc


def _scan_body(nc, ht_pool, xpool, xtpool, xapool, hnat_pool,
               ps_pre, ps_tr, ps_big,
               MT_s, WxT_s, ab_bc, ident, hT_cur,
               x_d, y_d, n_blocks, rowtiles_per_blk):
    if True:
        for blk in range(n_blocks):
            # ---- xa precompute for this block of 64 timesteps ----
            xa_tiles = []
            for sb in range(rowtiles_per_blk):
                row0 = (blk * T_BLK + sb * STEPS_PER_ROWTILE) * BL
                xt = xpool.tile([P, I], F32)
                nc.sync.dma_start(xt[:], x_d[row0:row0 + P, :])
                xT = xtpool.tile([P, IC, P], F32)
                for ic in range(IC):
                    pt = ps_big.tile([P, P], F32, tag="big")
                    nc.tensor.transpose(pt[:], xt[:, ic * P:(ic + 1) * P], ident[:])
                    nc.vector.tensor_copy(xT[:, ic, :], pt[:])
                xa_t = xapool.tile([P, H], F32)
                for nh in range(NHALF):
                    pp = ps_big.tile([P, 512], F32, tag="big")
                    for ic in range(IC):
                        nc.tensor.matmul(
                            pp[:, :],
                            xT[:, ic, :],
                            WxT_s[:, ic, nh * 512:(nh + 1) * 512],
                            start=(ic == 0),
                            stop=(ic == IC - 1),
                        )
                    # evict psum -> sbuf, adding the (broadcast) bias
                    nc.vector.tensor_tensor(
                        xa_t[:, nh * 512:(nh + 1) * 512],
                        pp[:, :],
                        ab_bc[:, nh * 512:(nh + 1) * 512],
                        mybir.AluOpType.add,
                    )
                xa_tiles.append(xa_t)

            # ---- the sequential scan over 64 timesteps ----
            for tt in range(T_BLK):
                t = blk * T_BLK + tt
                xa_t = xa_tiles[tt // STEPS_PER_ROWTILE]
                r0 = (tt % STEPS_PER_ROWTILE) * BL

                pss = []
                for nh in range(NHALF):
                    ps = ps_pre.tile([BL, 512], F32, tag="pre")
                    for c in range(KC):
                        nc.tensor.matmul(
                            ps[:, :],
                            hT_cur[:, c * BL:(c + 1) * BL],
                            MT_s[:, c, nh * 512:(nh + 1) * 512],
                            start=(c == 0),
                            stop=False,
                        )
                    # inject xa row-block via selector matmul:
                    # out[m,n] += sum_p I[p, r0+m] * xa[p, n] = xa[r0+m, n]
                    nc.tensor.matmul(
                        ps[:, :],
                        ident[:, r0:r0 + BL],
                        xa_t[:, nh * 512:(nh + 1) * 512],
                        start=False,
                        stop=True,
                    )
                    pss.append(ps)

                h_nat = hnat_pool.tile([BL, H], F32)
                for nh in range(NHALF):
                    nc.scalar.activation(
                        h_nat[:, nh * 512:(nh + 1) * 512],
                        pss[nh][:, :],
                        mybir.ActivationFunctionType.Relu,
                    )

                # transpose h_nat back into hT layout for the next step
                pt = ps_tr.tile([P, KC * BL], F32, tag="tr")
                for c in range(KC):
                    nc.tensor.transpose(
                        pt[:, c * BL:(c + 1) * BL],
                        h_nat[:, c * P:(c + 1) * P],
                        ident[:BL, :BL],
                    )
                hT_new = ht_pool.tile([P, KC * BL], F32)
                nc.vector.tensor_copy(hT_new[:], pt[:])

                nc.sync.dma_start(y_d[t * BL:(t + 1) * BL, :], h_nat[:])
                hT_cur = hT_new


def _host_prep(x, h0, W, b):
    """Shard + pre-transform inputs for the 8 cores."""
    a = np.float32(ALPHA)
    Wx = W[:, :I]                    # (H, I)
    Wh = W[:, I:]                    # (H, H)
    MT = ((1.0 - a) * np.eye(H, dtype=np.float32) + a * Wh).T.copy()  # (H, H) = M'^T
    WxT = (a * Wx).T.copy()          # (I, H)
    ab = (a * b).astype(np.float32)  # (H,)

    in_maps = []
    for core in range(NCORES):
        bs = slice(core * BL, (core + 1) * BL)
        x_local = np.ascontiguousarray(x[:, bs, :]).reshape(T * BL, I)
        h0T = np.ascontiguousarray(h0[bs, :].T)  # (H, BL)
        in_maps.append({
            "x": x_local.astype(np.float32),
            "h0T": h0T.astype(np.float32),
            "MT": MT,
            "WxT": WxT,
            "ab": ab,
        })
    return in_maps


def run(x, h0, W, b, trace=False, **spmd_kwargs):
    x = np.asarray(x, dtype=np.float32)
    h0 = np.asarray(h0, dtype=np.float32)
    W = np.asarray(W, dtype=np.float32)
    b = np.asarray(b, dtype=np.float32)

    nc = build_nc(T)
    in_maps = _host_prep(x, h0, W, b)
    res = run_bass_kernel_spmd(
        nc, in_maps, list(range(NCORES)), trace=trace, **spmd_kwargs
    )

    y = np.empty((T, B, H), dtype=np.float32)
    for core in range(NCORES):
        y[:, core * BL:(core + 1) * BL, :] = res.results[core]["y"].reshape(T, BL, H)
    h_final = y[-1].copy()
    return y, h_final, res


def kernel(x, h0, W, b):
    y, h_final, _ = run(x, h0, W, b)
    return y, h_final


# revision 11
# speedup vs baseline: 15.6419x; 2.6629x over previous
"""CTRNN Bass/Tile kernel for Trainium2 (8 NeuronCores, SPMD data-parallel over batch).

Problem: h_t = ReLU((1-a)*h + a*(W @ [x_t, h] + b)),  T=512, B=64, I=256, H=1024.

Strategy (per core, B_local = 8 batches):
  - Host prep: MT = (a*Wh)^T, WxT = a*Wx^T, ab = a*b, h0T = h0^T, g0 = (1-a)*h0.
  - Device: xa = x @ WxT + ab precomputed per 64-step block with PE GEMMs
    (x transposed on-chip via PE transpose), stored fp32r.
  - Scan, per step (h kept TRANSPOSED in SBUF as hT, fp32r; decay term kept
    exact in fp32 via the scalar engine):
      psum(8,512) x2 = sum_c hT[:,c].T @ MT[:,c,half]      (fp32r, 1 cyc/col)
                      + selector-matmul xa row-block        (fp32r)
      psum += g_prev (= (1-a)*h_prev, exact fp32)           (DVE add)
      h_nat = ReLU(psum)                                    (DVE / ACT)
      g_new = ReLU((1-a)*psum) = (1-a)*h_nat                (ACT, scale pre-func)
      8 PE transposes h_nat -> hT_new (fp32 -> fp32r copy)
      DMA h_nat -> y[t]
  fp32r streams the moving operand at bf16 rate with ~1e-4 relative error;
  keeping the (1-a)*h passthrough in fp32 avoids compounding that error
  through the 512-step recurrence.
"""

import numpy as np
from contextlib import ExitStack

import concourse.bass as bass
import concourse.bacc as bacc
import concourse.mybir as mybir
import concourse.tile as tile
from concourse.bass_utils import run_bass_kernel_spmd
from concourse.masks import make_identity

F32 = mybir.dt.float32
F32R = mybir.dt.float32r

T, B, I, H = 512, 64, 256, 1024
ALPHA = 10.0 / 100.0
NCORES = 8
BL = B // NCORES          # 8 batches per core
P = 128
KC = H // P               # 8 hidden chunks
IC = I // P               # 2 input chunks
NHALF = 2                 # 1024 = 2 x 512 (psum bank limit)
STEPS_PER_ROWTILE = P // BL   # 16 timesteps per 128-row tile
T_BLK = 64                # timesteps per xa block (4 row-tiles of 128)


def build_nc(t_total: int = T, repeat: int = 1) -> bass.Bass:
    assert t_total % T_BLK == 0
    nc = bacc.Bacc()

    x_d = nc.dram_tensor("x", [t_total * BL, I], F32, kind="ExternalInput")
    h0T_d = nc.dram_tensor("h0T", [H, BL], F32R, kind="ExternalInput")
    g0_d = nc.dram_tensor("g0", [BL, H], F32, kind="ExternalInput")
    MT_d = nc.dram_tensor("MT", [H, H], F32R, kind="ExternalInput")
    WxT_d = nc.dram_tensor("WxT", [I, H], F32, kind="ExternalInput")
    ab_d = nc.dram_tensor("ab", [H], F32, kind="ExternalInput")
    y_d = nc.dram_tensor("y", [t_total * BL, H], F32, kind="ExternalOutput")

    with ExitStack() as ctx:
        tc = ctx.enter_context(tile.TileContext(nc))
        consts = ctx.enter_context(tc.tile_pool(name="consts", bufs=1))
        xpool = ctx.enter_context(tc.tile_pool(name="xpool", bufs=3))
        xtpool = ctx.enter_context(tc.tile_pool(name="xtpool", bufs=3))
        xapool = ctx.enter_context(tc.tile_pool(name="xapool", bufs=8))
        hnat_pool = ctx.enter_context(tc.tile_pool(name="hnat", bufs=4))
        g_pool = ctx.enter_context(tc.tile_pool(name="gpool", bufs=2))
        ht_pool = ctx.enter_context(tc.tile_pool(name="ht", bufs=2))
        ps_pre = ctx.enter_context(tc.tile_pool(name="ps_pre", bufs=4, space="PSUM"))
        ps_tr = ctx.enter_context(tc.tile_pool(name="ps_tr", bufs=2, space="PSUM"))
        ps_big = ctx.enter_context(tc.tile_pool(name="ps_big", bufs=2, space="PSUM"))

        # ---- persistent constants ----
        MT_s = consts.tile([P, KC, H], F32R)
        nc.sync.dma_start(MT_s[:], MT_d[:, :].rearrange("(c p) j -> p c j", p=P))
        WxT_s = consts.tile([P, IC, H], F32)
        nc.sync.dma_start(WxT_s[:], WxT_d[:, :].rearrange("(c p) j -> p c j", p=P))
        ab_bc = consts.tile([P, H], F32)
        nc.sync.dma_start(ab_bc[:], ab_d[None, :].to_broadcast((P, H)))
        ident = consts.tile([P, P], F32)
        make_identity(nc, ident[:])
        identr = consts.tile([P, P], F32R)
        nc.vector.tensor_copy(identr[:], ident[:])

        n_blocks = t_total // T_BLK
        rowtiles_per_blk = T_BLK // STEPS_PER_ROWTILE  # 4

        for rep in range(repeat):
            hT_cur = ht_pool.tile([P, KC * BL], F32R)
            nc.sync.dma_start(
                hT_cur[:].rearrange("p (c b) -> p c b", c=KC),
                h0T_d[:, :].rearrange("(c p) b -> p c b", p=P),
            )
            g_cur = g_pool.tile([BL, H], F32)
            nc.sync.dma_start(g_cur[:], g0_d[:, :])
            _scan_body(nc, ht_pool, xpool, xtpool, xapool, hnat_pool, g_pool,
                       ps_pre, ps_tr, ps_big,
                       MT_s, WxT_s, ab_bc, ident, identr, hT_cur, g_cur,
                       x_d, y_d, n_blocks, rowtiles_per_blk)

    if not nc.is_finalized():
        nc.finalize()
    return nc


def _scan_body(nc, ht_pool, xpool, xtpool, xapool, hnat_pool, g_pool,
               ps_pre, ps_tr, ps_big,
               MT_s, WxT_s, ab_bc, ident, identr, hT_cur, g_cur,
               x_d, y_d, n_blocks, rowtiles_per_blk):
    OMA = float(1.0 - ALPHA)
    for blk in range(n_blocks):
        # ---- xa precompute for this block of 64 timesteps (fp32 GEMM) ----
        xa_tiles = []
        for sb in range(rowtiles_per_blk):
            row0 = (blk * T_BLK + sb * STEPS_PER_ROWTILE) * BL
            xt = xpool.tile([P, I], F32)
            nc.sync.dma_start(xt[:], x_d[row0:row0 + P, :])
            xT = xtpool.tile([P, IC, P], F32)
            for ic in range(IC):
                pt = ps_big.tile([P, P], F32, tag="big")
                nc.tensor.transpose(pt[:], xt[:, ic * P:(ic + 1) * P], ident[:])
                nc.vector.tensor_copy(xT[:, ic, :], pt[:])
            xa_t = xapool.tile([P, H], F32R)
            for nh in range(NHALF):
                pp = ps_big.tile([P, 512], F32, tag="big")
                for ic in range(IC):
                    nc.tensor.matmul(
                        pp[:, :],
                        xT[:, ic, :],
                        WxT_s[:, ic, nh * 512:(nh + 1) * 512],
                        start=(ic == 0),
                        stop=(ic == IC - 1),
                    )
                # evict psum -> sbuf (fp32r bits), adding the broadcast bias
                nc.vector.tensor_tensor(
                    xa_t[:, nh * 512:(nh + 1) * 512],
                    pp[:, :],
                    ab_bc[:, nh * 512:(nh + 1) * 512],
                    mybir.AluOpType.add,
                )
            xa_tiles.append(xa_t)

        # ---- the sequential scan over 64 timesteps ----
        for tt in range(T_BLK):
            t = blk * T_BLK + tt
            xa_t = xa_tiles[tt // STEPS_PER_ROWTILE]
            r0 = (tt % STEPS_PER_ROWTILE) * BL

            pss = []
            for nh in range(NHALF):
                ps = ps_pre.tile([BL, 512], F32, tag="pre")
                for c in range(KC):
                    nc.tensor.matmul(
                        ps[:, :],
                        hT_cur[:, c * BL:(c + 1) * BL],
                        MT_s[:, c, nh * 512:(nh + 1) * 512],
                        start=(c == 0),
                        stop=False,
                    )
                # inject xa row-block via selector matmul:
                # out[m,n] += sum_p I[p, r0+m] * xa[p, n] = xa[r0+m, n]
                nc.tensor.matmul(
                    ps[:, :],
                    identr[:, r0:r0 + BL],
                    xa_t[:, nh * 512:(nh + 1) * 512],
                    start=False,
                    stop=True,
                )
                pss.append(ps)

            h_nat = hnat_pool.tile([BL, H], F32)
            g_new = g_pool.tile([BL, H], F32)
            for nh in range(NHALF):
                sl = slice(nh * 512, (nh + 1) * 512)
                # exact fp32 decay passthrough: psum += (1-a)*h_prev
                nc.vector.tensor_tensor(
                    pss[nh][:, :], pss[nh][:, :], g_cur[:, sl],
                    mybir.AluOpType.add,
                )
                # h = relu(pre)  (DVE), g = (1-a)*h = relu((1-a)*pre)  (ACT)
                nc.vector.tensor_scalar_max(h_nat[:, sl], pss[nh][:, :], 0.0)
                nc.scalar.activation(
                    g_new[:, sl], pss[nh][:, :],
                    mybir.ActivationFunctionType.Relu, scale=OMA,
                )

            # transpose h_nat back into hT layout for the next step
            pt = ps_tr.tile([P, KC * BL], F32, tag="tr")
            for c in range(KC):
                nc.tensor.transpose(
                    pt[:, c * BL:(c + 1) * BL],
                    h_nat[:, c * P:(c + 1) * P],
                    ident[:BL, :BL],
                )
            hT_new = ht_pool.tile([P, KC * BL], F32R)
            nc.vector.tensor_copy(hT_new[:], pt[:])

            nc.sync.dma_start(y_d[t * BL:(t + 1) * BL, :], h_nat[:])
            hT_cur = hT_new
            g_cur = g_new


def _host_prep(x, h0, W, b):
    """Shard + pre-transform inputs for the 8 cores."""
    a = np.float32(ALPHA)
    Wx = W[:, :I]                    # (H, I)
    Wh = W[:, I:]                    # (H, H)
    MT = np.ascontiguousarray((a * Wh).T)   # (H, H) = (a*Wh)^T
    WxT = np.ascontiguousarray((a * Wx).T)  # (I, H)
    ab = (a * b).astype(np.float32)  # (H,)

    in_maps = []
    for core in range(NCORES):
        bs = slice(core * BL, (core + 1) * BL)
        x_local = np.ascontiguousarray(x[:, bs, :]).reshape(T * BL, I)
        h0_local = np.ascontiguousarray(h0[bs, :])
        in_maps.append({
            "x": x_local.astype(np.float32),
            "h0T": np.ascontiguousarray(h0_local.T),
            "g0": ((1.0 - a) * h0_local).astype(np.float32),
            "MT": MT,
            "WxT": WxT,
            "ab": ab,
        })
    return in_maps


def run(x, h0, W, b, trace=False, **spmd_kwargs):
    x = np.asarray(x, dtype=np.float32)
    h0 = np.asarray(h0, dtype=np.float32)
    W = np.asarray(W, dtype=np.float32)
    b = np.asarray(b, dtype=np.float32)

    nc = build_nc(T)
    in_maps = _host_prep(x, h0, W, b)
    res = run_bass_kernel_spmd(
        nc, in_maps, list(range(NCORES)), trace=trace, **spmd_kwargs
    )

    y = np.empty((T, B, H), dtype=np.float32)
    for core in range(NCORES):
        y[:, core * BL:(core + 1) * BL, :] = res.results[core]["y"].reshape(T, BL, H)
    h_final = y[-1].copy()
    return y, h_final, res


def kernel(x, h0, W, b):
    y, h_final, _ = run(x, h0, W, b)
    return y, h_final
